# revision 5
# baseline (speedup 1.0000x reference)
"""BioDecoder teacher-forcing kernel for 8 Trainium2 NeuronCores (Bass/Tile).

Strategy (data-parallel over batch B=8, one batch element per core):
  - embedding lookup via indirect DMA gather + PE transpose
  - LSTM input projections W_ih @ x matmul'd DIRECTLY into the gates PSUM
    region once per 32-step chunk (start=True), biases injected via a
    1-partition ones-row matmul; the per-step work is only the 16 hh
    matmuls accumulating on top (start=False)
  - 2-layer recurrence wavefront (layer1 lags layer0 by one chunk);
    per layer-step chain: PE(16mm) -> ACT sigmoid -> {DVE u || Pool v}
    -> DVE c -> ACT sigmoid(2c) -> DVE h, all transcendentals via
    Sigmoid only (tanh(x) = 2*sigmoid(2x)-1)
  - gate MLP + output projection (vocab x hidden, fp16) chunked over
    time, emitted inside the wavefront so the scheduler fills PE gaps
  - logits staged to fp16 SBUF in [128,1024] pairs and DMA'd out on the
    two HWDGE queues (sync + scalar) alternately; host concatenates

Self-contained: hardcodes all shapes from the problem spec.
"""

import os
import numpy as np

import concourse.bacc as bacc
import concourse.bass as bass
import concourse.mybir as mybir
import concourse.tile as tile
from concourse.bass import IndirectOffsetOnAxis
from concourse.bass_utils import run_bass_kernel_spmd
from concourse.dve_ops import AFFINE_MUL_REDUCE
from concourse.masks import make_identity

F16 = mybir.dt.float16
F32 = mybir.dt.float32
I32 = mybir.dt.int32
AF = mybir.ActivationFunctionType
OP = mybir.AluOpType

VOCAB, EMBED, HIDDEN = 32000, 128, 256
B, T = 8, 512
TT = T - 1          # 511 recurrence steps
NM = 8              # gate M-tiles (4*HIDDEN / 128)
NK = 2              # hidden K-tiles (HIDDEN / 128)
CH = 32             # gates-psum chunk size (steps)
LAG = CH            # layer-1 lag behind layer-0
TCH = 128           # output-projection time chunk
VN = 512            # vocab tile (one PSUM bank of fp32)
N_CORES = 8

# gate reorder: pytorch i,f,g,o  ->  i,f,o,g (so sigmoid gates are contiguous)
PERM = np.r_[0:256, 256:512, 768:1024, 512:768]


def _chunks(tsteps, step):
    out = []
    s = 0
    while s < tsteps:
        e = min(s + step, tsteps)
        out.append((s, e))
        s = e
    return out


def build_program(tsteps=TT):
    """Emit the full SPMD program; returns compiled nc."""
    nc = bacc.Bacc("TRN2", target_bir_lowering=False, debug=False,
                   enable_asserts=False, num_devices=N_CORES)

    cap_d = nc.dram_tensor("cap", [128, 4], I32, kind="ExternalInput")
    emb_d = nc.dram_tensor("emb", [VOCAB, EMBED], F16, kind="ExternalInput")
    h0_d = nc.dram_tensor("h0", [128, NK], F16, kind="ExternalInput")
    whh0_d = nc.dram_tensor("whh0", [128, NK * 1024], F16, kind="ExternalInput")
    whh1_d = nc.dram_tensor("whh1", [128, NK * 1024], F16, kind="ExternalInput")
    wih0_d = nc.dram_tensor("wih0", [128, 1024], F16, kind="ExternalInput")
    wih1_d = nc.dram_tensor("wih1", [128, NK * 1024], F16, kind="ExternalInput")
    br_d = nc.dram_tensor("br", [1, 2048], F16, kind="ExternalInput")
    gw1_d = nc.dram_tensor("gw1", [128, 512], F16, kind="ExternalInput")
    gw2_d = nc.dram_tensor("gw2", [128, NK], F16, kind="ExternalInput")
    gb1_d = nc.dram_tensor("gb1", [128, 2], F32, kind="ExternalInput")
    gb2_d = nc.dram_tensor("gb2", [1, 1], F32, kind="ExternalInput")
    outw_d = nc.dram_tensor("outw", [HIDDEN, VOCAB], F16, kind="ExternalInput")
    logits_d = nc.dram_tensor("logits", [tsteps, VOCAB], F16,
                              kind="ExternalOutput")

    n_gchunks = (tsteps + 127) // 128  # embedding gather chunks

    from contextlib import ExitStack
    with tile.TileContext(nc) as tc, ExitStack() as ctx:
        const = ctx.enter_context(tc.tile_pool(name="const", bufs=1))
        sp = ctx.enter_context(tc.tile_pool(name="sp", bufs=4))
        gp = ctx.enter_context(tc.tile_pool(name="gp", bufs=2))
        stp = ctx.enter_context(tc.tile_pool(name="stp", bufs=3))
        pg0 = ctx.enter_context(tc.tile_pool(name="pg0", bufs=2, space="PSUM"))
        pg1 = ctx.enter_context(tc.tile_pool(name="pg1", bufs=2, space="PSUM"))
        pbig = ctx.enter_context(tc.tile_pool(name="pbig", bufs=3, space="PSUM"))

        # ---- persistent SBUF buffers ----
        whh0 = const.tile([128, NK * 1024], F16)
        whh1 = const.tile([128, NK * 1024], F16)
        wih0 = const.tile([128, 1024], F16)
        wih1 = const.tile([128, NK * 1024], F16)
        br = const.tile([1, 2048], F16)   # [b0 row | b1 row]
        gw1 = const.tile([128, 512], F16)
        gw2 = const.tile([128, NK], F16)
        gb1 = const.tile([128, 2], F32)
        gb2 = const.tile([1, 1], F32)
        h0 = const.tile([128, NK], F16)
        idx = const.tile([128, 4], I32)
        ident = const.tile([128, 128], F16)
        ones = const.tile([1, 128], F16)
        xT = const.tile([128, n_gchunks * 128], F16)
        H1 = const.tile([128, tsteps, NK], F16)
        H2 = const.tile([128, tsteps, NK], F16)
        outw = const.tile([128, NK, VOCAB], F16)

        for dst, src in ((whh0, whh0_d), (whh1, whh1_d), (wih0, wih0_d),
                         (wih1, wih1_d), (br, br_d),
                         (gw1, gw1_d), (gw2, gw2_d), (gb1, gb1_d),
                         (gb2, gb2_d), (h0, h0_d), (idx, cap_d)):
            nc.sync.dma_start(out=dst[:, :], in_=src[:, :])
        # outw: [hidden(2*128), vocab] -> sbuf [128, ki, vocab]; split the
        # 16MB load across both HWDGE queues
        half = VOCAB // 2
        for ki in range(NK):
            nc.sync.dma_start(out=outw[:, ki, 0:half],
                              in_=outw_d[ki * 128:(ki + 1) * 128, 0:half])
            nc.scalar.dma_start(out=outw[:, ki, half:VOCAB],
                                in_=outw_d[ki * 128:(ki + 1) * 128, half:VOCAB])
        make_identity(nc, ident[:, :])
        nc.vector.memset(ones[:, :], 1.0)

        # ---- embedding gather + transpose ----
        for j in range(n_gchunks):
            xg = sp.tile([128, 128], F16, tag="xg")
            nc.gpsimd.indirect_dma_start(
                out=xg[:, :], out_offset=None,
                in_=emb_d[:, :],
                in_offset=IndirectOffsetOnAxis(ap=idx[:, j:j + 1], axis=0),
            )
            tp = pbig.tile([128, 512], F16, tag="pb")
            nc.tensor.transpose(tp[:, 0:128], xg[:, :], ident[:, :])
            nc.scalar.copy(xT[:, j * 128:(j + 1) * 128], tp[:, 0:128])

        # ---- wavefront state ----
        c_prev = [None, None]
        whh = [whh0, whh1]
        Hbuf = [H1, H2]
        gpsum = [{}, {}]   # chunk-start -> psum tile, per layer
        st = [{}, {}]      # per-layer in-flight step state
        pools = [pg0, pg1]

        def inject_chunk(L, cs, ce):
            """xp (+bias) for steps [cs, ce) matmul'd into a fresh gates-psum
            chunk tile; per-step hh matmuls then accumulate on top."""
            n = ce - cs
            ps = pools[L].tile([128, NM, CH], F32, tag=f"gp{L}")
            # start=True marks the tile's whole PSUM bank pending-zero; it
            # must appear EXACTLY ONCE per chunk tile (first matmul), or
            # later starts wipe earlier m-tiles' contributions.
            first = [True]

            def mm(out, lhsT, rhs):
                nc.tensor.matmul(out, lhsT, rhs, start=first[0], stop=False)
                first[0] = False

            if L == 0:
                for m in range(NM):
                    mm(ps[:, m, 0:n], wih0[:, m * 128:(m + 1) * 128],
                       xT[:, cs:ce])
            else:
                for m in range(NM):
                    for ki in range(NK):
                        mm(ps[:, m, 0:n],
                           wih1[:, ki * 1024 + m * 128: ki * 1024 + (m + 1) * 128],
                           H1[:, cs:ce, ki])
            # bias row: out[p, f] += b[p] * ones[f]  (k=1 matmul)
            boff = L * 1024
            for m in range(NM):
                nc.tensor.matmul(ps[:, m, 0:n],
                                 br[0:1, boff + m * 128: boff + (m + 1) * 128],
                                 ones[0:1, 0:n], start=False, stop=False)
            gpsum[L][cs] = ps

        # per-step stages; g-gate pre-activations are pre-scaled by 2 on the
        # host so a single sigmoid covers all gates (tanh(g) = 2*sig(2g)-1)
        def stage_mm(L, t):
            h_ap = h0[:, :] if t == 0 else Hbuf[L][:, t - 1, :]
            cs = (t // CH) * CH
            ps = gpsum[L][cs]
            ti = t - cs
            for m in range(NM):
                for ki in range(NK):
                    nc.tensor.matmul(
                        ps[:, m, ti:ti + 1],
                        whh[L][:, ki * 1024 + m * 128: ki * 1024 + (m + 1) * 128],
                        h_ap[:, ki:ki + 1],
                        start=False, stop=(ki == NK - 1))
            st[L]["ps"] = ps
            st[L]["ti"] = ti

        def stage_act1(L, t):
            # a: [sig_i(2) sig_f(2) sig_o(2) sig_2g(2)]
            a = sp.tile([128, NM], F32, tag=f"a{L}")
            nc.scalar.activation(a[:, :], st[L]["ps"][:, :, st[L]["ti"]],
                                 AF.Sigmoid)
            st[L]["a"] = a

        def stage_v(L, t):
            # forget path: f * c_prev (GpSimd is too slow per-op on TRN2,
            # so this stays on DVE right after act1)
            if c_prev[L] is not None:
                v = sp.tile([128, NK], F32, tag=f"v{L}")
                nc.vector.tensor_mul(v[:, :], st[L]["a"][:, 2:4],
                                     c_prev[L][:, :])
                st[L]["v"] = v

        def stage_u(L, t):
            # u = sig_i * tanh(g) = (2*sig(2g) - 1) * sig_i   (one fused op)
            u = sp.tile([128, NK], F32, tag=f"u{L}")
            nc.vector._custom_dve(AFFINE_MUL_REDUCE, out=u[:, :],
                                  in0=st[L]["a"][:, 6:8], in1=st[L]["a"][:, 0:2],
                                  s0=2.0, s1=-1.0)
            st[L]["u"] = u

        def stage_c(L, t):
            if c_prev[L] is None:
                c_prev[L] = st[L]["u"]
            else:
                c = sp.tile([128, NK], F32, tag=f"c{L}")
                nc.vector.tensor_add(c[:, :], st[L]["u"][:, :], st[L]["v"][:, :])
                c_prev[L] = c

        def stage_act2(L, t):
            sc = sp.tile([128, NK], F32, tag=f"sc{L}")
            nc.scalar.activation(sc[:, :], c_prev[L][:, :], AF.Sigmoid, scale=2.0)
            st[L]["sc"] = sc

        def stage_h(L, t):
            # h = sig_o * tanh(c) = (2*sig(2c) - 1) * sig_o   (one fused op)
            nc.vector._custom_dve(AFFINE_MUL_REDUCE, out=Hbuf[L][:, t, :],
                                  in0=st[L]["sc"][:, :], in1=st[L]["a"][:, 4:6],
                                  s0=2.0, s1=-1.0)

        STAGES = (stage_mm, stage_act1, stage_v, stage_u, stage_c,
                  stage_act2, stage_h)

        def out_chunk(ts_, te_):
            nt = te_ - ts_
            # t1 = sig(2*(H2 @ gw1.T + gb1))   (tanh folded into gw2/gb2 host-side)
            t1 = gp.tile([128, NK, TCH], F16, tag="t1")
            for mi in range(2):
                ps = pbig.tile([128, 512], F32, tag="pb")
                for ki in range(NK):
                    nc.tensor.matmul(
                        ps[:, 0:nt],
                        gw1[:, ki * 256 + mi * 128: ki * 256 + (mi + 1) * 128],
                        H2[:, ts_:te_, ki],
                        start=(ki == 0), stop=(ki == NK - 1))
                nc.scalar.activation(t1[:, mi, 0:nt], ps[:, 0:nt], AF.Sigmoid,
                                     bias=gb1[:, mi:mi + 1], scale=2.0)
            psg = pbig.tile([128, 512], F32, tag="pb")
            for ki in range(NK):
                nc.tensor.matmul(psg[0:1, 0:nt], gw2[:, ki:ki + 1],
                                 t1[:, ki, 0:nt],
                                 start=(ki == 0), stop=(ki == NK - 1))
            g16 = gp.tile([1, TCH], F16, tag="g16")
            nc.scalar.activation(g16[0:1, 0:nt], psg[0:1, 0:nt], AF.Sigmoid,
                                 bias=gb2[0:1, 0:1])
            bc = pbig.tile([128, 512], F32, tag="pb")
            nc.tensor.matmul(bc[:, 0:nt], ones[0:1, :], g16[0:1, 0:nt],
                             start=True, stop=True)
            gated = gp.tile([128, NK, TCH], F16, tag="gated")
            for ki in range(NK):
                nc.vector.tensor_mul(gated[:, ki, 0:nt], H2[:, ts_:te_, ki],
                                     bc[:, 0:nt])
            # logits: fp16 staging pairs -> alternate the two HWDGE queues
            nvt = (VOCAB + VN - 1) // VN
            stage = None
            for vt in range(nvt):
                v0 = vt * VN
                nv = min(VN, VOCAB - v0)
                ps = pbig.tile([128, 512], F32, tag="pb")
                for ki in range(NK):
                    nc.tensor.matmul(ps[0:nt, 0:nv], gated[:, ki, 0:nt],
                                     outw[:, ki, v0:v0 + nv],
                                     start=(ki == 0), stop=(ki == NK - 1))
                k = vt % 2
                if k == 0:
                    stage = stp.tile([128, 2 * VN], F16, tag="lg")
                nc.vector.tensor_copy(stage[0:nt, k * VN:k * VN + nv],
                                      ps[0:nt, 0:nv])
                if k == 1 or vt == nvt - 1:
                    w = k * VN + nv
                    eng = nc.sync if (vt // 2) % 2 == 0 else nc.scalar
                    eng.dma_start(out=logits_d[ts_:te_, vt // 2 * 2 * VN:
                                               vt // 2 * 2 * VN + w],
                                  in_=stage[0:nt, 0:w])

        xpc = _chunks(tsteps, CH)
        tch = _chunks(tsteps, TCH)
        # layer-0 chunk [cs,ce) injected one slot before first use; layer-1
        # chunk injected right after H1[ce-1] lands (end of slot ce-1)
        inj0 = {cs - 1: (cs, ce) for cs, ce in xpc}
        inj1 = {ce - 1: (cs, ce) for cs, ce in xpc}
        tci = {te - 1: (ts_, te) for ts_, te in tch}

        inject_chunk(0, 0, CH)  # prologue: first layer-0 chunk
        for t in range(tsteps + LAG):
            s = t - LAG
            # interleave the two layers' chains stage-by-stage so each
            # engine's FIFO alternates between the independent chains
            for f in STAGES:
                if t < tsteps:
                    f(0, t)
                if 0 <= s < tsteps:
                    f(1, s)
            if t < tsteps and t in inj0:
                inject_chunk(0, *inj0[t])
            if t < tsteps and t in inj1:
                inject_chunk(1, *inj1[t])
            if 0 <= s < tsteps and s in tci:
                with tc.high_priority(offset=-3000):
                    out_chunk(*tci[s])

    nc.compile()
    return nc


def prep_inputs(inputs, tsteps=TT):
    """Host-side: permute/tile/cast weights, build per-core in_maps."""
    g = {k: np.asarray(v) for k, v in inputs.items()}

    def f16(x):
        return np.ascontiguousarray(x.astype(np.float16))

    def gate_scale(wp):
        # pre-scale the g-gate block (post-perm rows 768:1024) by 2 so that
        # sigmoid(pre) directly yields sig(2g) for the tanh identity
        wp = wp.copy()
        wp[768:1024] *= 2.0
        return wp

    def tile_whh(w):  # [1024, 256] -> [128, ki*1024 + m*128 + j]
        wp = gate_scale(w[PERM].astype(np.float32))
        return f16(wp.reshape(8, 128, 2, 128).transpose(3, 2, 0, 1)
                   .reshape(128, 2048))

    def tile_wih0(w):  # [1024, 128] -> [128(e), m*128 + j]
        wp = gate_scale(w[PERM].astype(np.float32))
        return f16(wp.reshape(8, 128, 128).transpose(2, 0, 1).reshape(128, 1024))

    whh0 = tile_whh(g["w_hh_l0"])
    whh1 = tile_whh(g["w_hh_l1"])
    wih0 = tile_wih0(g["w_ih_l0"])
    wih1 = tile_whh(g["w_ih_l1"])     # same [1024, 256] layout

    bp0 = gate_scale((g["b_ih_l0"] + g["b_hh_l0"])[PERM].astype(np.float32))
    bp1 = gate_scale((g["b_ih_l1"] + g["b_hh_l1"])[PERM].astype(np.float32))
    br = f16(np.concatenate([bp0, bp1]).reshape(1, 2048))

    gw1 = f16(g["gate_w1"].astype(np.float32).reshape(2, 128, 2, 128)
              .transpose(3, 2, 0, 1).reshape(128, 512))
    # t1 is stored as sigmoid(2x); tanh = 2*t1-1 folded into gw2/gb2:
    #   gate pre-act = gw2 @ (2*t1-1) + gb2 = (2*gw2) @ t1 + (gb2 - sum(gw2))
    gw2v = g["gate_w2"].astype(np.float32).reshape(256)
    gw2 = f16((2.0 * gw2v).reshape(2, 128).T)
    gb2 = np.array([[g["gate_b2"].astype(np.float32).reshape(()) - gw2v.sum()]],
                   dtype=np.float32)
    gb1 = np.ascontiguousarray(
        (2.0 * g["gate_b1"].astype(np.float32)).reshape(2, 128).T)

    emb = f16(g["emb_w"])
    outw = f16(g["out_w"].astype(np.float32).T)       # [256, 32000]

    caps = np.asarray(g["captions"], dtype=np.int32)  # [B, T]
    thought = g["thought"].astype(np.float32)          # [B, 256]

    n_gchunks = (tsteps + 127) // 128
    in_maps = []
    for b in range(B):
        capb = np.zeros((128, 4), dtype=np.int32)
        toks = caps[b, :tsteps]
        for j in range(n_gchunks):
            seg = toks[j * 128:(j + 1) * 128]
            capb[:len(seg), j] = seg
        h0 = f16(thought[b].reshape(2, 128).T)
        in_maps.append({
            "cap": capb, "emb": emb, "h0": h0,
            "whh0": whh0, "whh1": whh1, "wih0": wih0, "wih1": wih1,
            "br": br, "gw1": gw1, "gw2": gw2,
            "gb1": gb1, "gb2": gb2, "outw": outw,
        })
    return in_maps


_cached = {}


def _get_program(tsteps=TT):
    if tsteps not in _cached:
        _cached[tsteps] = build_program(tsteps)
    return _cached[tsteps]


def kernel(**inputs) -> np.ndarray:
    tsteps = int(os.environ.get("BIODEC_T", TT))
    nc = _get_program(tsteps)
    in_maps = prep_inputs(inputs, tsteps)
    res = run_bass_kernel_spmd(nc, in_maps, list(range(N_CORES)))
    out = np.stack([res.results[b]["logits"].astype(np.float32)
                    for b in range(B)], axis=0)
    out_b = np.asarray(inputs["out_b"], dtype=np.float32)
    if np.any(out_b):
        out = out + out_b
    return out
